# revision 5
# baseline (speedup 1.0000x reference)
"""CircleLoss (nn_CircleLoss) Trainium2 kernel, 8-core SPMD, symmetric-triangle.

Strategy:
- Host: stable-sort rows by label, L2-normalize (fp64), cast bf16, transpose
  -> eTn [128, 8192]. Split rows into 16 blocks of 512. The 136 unordered
  block-pairs split evenly: each core gets 15 off-diagonal "pure" pairs,
  2 diagonal blocks, and up to 2 class-boundary correction windows. One
  compiled NEFF serves all 8 cores (pure SPMD; per-core work is steered
  entirely by host-gathered inputs eL/eR/masks). Diagonal slots sit at
  schedule positions 5 and 11 so their ACT-heavy band work interleaves with
  pure slots instead of tailing the program.
- Device per pure pair (i,j): 4 matmul chunks [128 anchors x 512 cols] of
  sim = eL^T eR; square via ACT-Square-from-PSUM ('E'/'A') or DVE-copy +
  Pool-square ('D'); exp(80 s^2 - 80) via ACT Exp with fused row-sum
  ('E', f32 path) or a bf16 Schraudolph bit-trick on DVE (i16 = A*sq + B,
  bitcast to bf16) at 4x DVE rate ('A'/'D'); mirror (column) sums via
  selector-matmuls accumulated into one PSUM bank (symmetry: s_ij serves
  rows of i and rows of j), trailing the sim stream by ~2 slots so the
  in-order PE never waits on the exp pipeline.
- Diagonal blocks: per anchor-tile a fixed 256-wide window (offsets
  0/64/192/256) covers all same-class columns (class size <= 64); window
  excluded from the dense exp and handled with masked sums: negatives
  Fb*negb, positives exp(80 (s-1)^2 - 162.8)*posm (ap = 80(s-1)^2 - 12.8,
  Schraudolph with an int16 clamp against bit-pattern underflow).
- Boundary corrections: classes straddling a 512-block edge produce
  same-class pairs inside pure blocks; a [128x128] window per boundary
  recomputes those sims, subtracts their F from the negative sums and adds
  the positive terms (rows directly, columns via selector colsums).
- Host: combines row partials + colsum planes + counts in fp64 into the
  final scalar.
"""

import numpy as np

_N, _D, _NCORES = 8192, 128, 8
_NB = 16                    # row blocks
_BS = 512                   # block size
_NPURE = 15                 # pure off-diag pairs per core
_NSLOT = 17                 # 15 pure + 2 diag
_W = 256                    # diag band window width
_W0 = (0, 64, 192, 256)     # diag window offset per anchor tile
_OUTW = 96
_CPOS = 150.0               # positives fixed shift: Ep = exp(80 usq - 162.8)

_A80 = 80.0 * 128.0 / np.log(2.0)          # schraudolph slope on sq
_BP = 127.0 * 128.0 - 4.3 - _A80           # schraudolph offset on sq
_AP80 = _A80                               # slope on usq (same 80x)
_BPP = 127.0 * 128.0 - 4.3 - (162.8 * 128.0 / np.log(2.0))  # offset for Ep

# schedule position of each slot; diag slots (15, 16) sit mid-stream
_SLOTORD = [0, 1, 2, 3, 4, 15, 5, 6, 7, 8, 9, 10, 11, 12, 13, 14, 16]
# corr slot s emitted after these schedule positions
_CORRPOS = {3: 0, 12: 1}


# per pure SLOT path: 'A' ACT-sq x4 + batched DVE schraudolph,
# 'D' DVE-copy x4 + batched Pool-sq + batched DVE schraudolph
# pair-level paths: slot k pairs (h=0,1) -> _PATHS[2k], _PATHS[2k+1]
_PATHS = "AA AD AA DA AA AD AA DA AA AD AA DA AA AA AA".replace(" ", "")

_cache = {}


def _build_nc(paths=_PATHS):
    from contextlib import ExitStack

    import concourse.bacc as bacc
    import concourse.mybir as mybir
    import concourse.tile as tile

    f32 = mybir.dt.float32
    bf16 = mybir.dt.bfloat16
    i16 = mybir.dt.int16
    OP = mybir.AluOpType
    AF = mybir.ActivationFunctionType

    nc = bacc.Bacc("TRN2", target_bir_lowering=False, debug=False,
                   num_devices=_NCORES)
    eL_d = nc.dram_tensor("eL", [128, _NSLOT * _BS], bf16, kind="ExternalInput").ap()
    eR_d = nc.dram_tensor("eR", [128, _NSLOT * _BS], bf16, kind="ExternalInput").ap()
    posmD_d = nc.dram_tensor("posmD", [128, 2 * 4 * _W], bf16, kind="ExternalInput").ap()
    negbD_d = nc.dram_tensor("negbD", [128, 2 * 4 * _W], bf16, kind="ExternalInput").ap()
    corrL_d = nc.dram_tensor("corrL", [128, 2 * 128], bf16, kind="ExternalInput").ap()
    corrR_d = nc.dram_tensor("corrR", [128, 2 * 128], bf16, kind="ExternalInput").ap()
    samec_d = nc.dram_tensor("samec", [128, 2 * 128], bf16, kind="ExternalInput").ap()
    outs_d = nc.dram_tensor("outs", [128, _OUTW], f32, kind="ExternalOutput").ap()
    colp_d = nc.dram_tensor("colp", [32, _BS], f32, kind="ExternalOutput").ap()

    with tile.TileContext(nc) as tc, ExitStack() as ctx:
        const = ctx.enter_context(tc.tile_pool(name="const", bufs=1))
        sqb = ctx.enter_context(tc.tile_pool(name="sqb", bufs=3))
        sqd = ctx.enter_context(tc.tile_pool(name="sqd", bufs=4))
        scp = ctx.enter_context(tc.tile_pool(name="scp", bufs=2))
        ipool = ctx.enter_context(tc.tile_pool(name="ipool", bufs=3))
        ibpool = ctx.enter_context(tc.tile_pool(name="ibpool", bufs=4))
        fpool = ctx.enter_context(tc.tile_pool(name="fpool", bufs=5))
        band = ctx.enter_context(tc.tile_pool(name="band", bufs=3))
        psum2 = ctx.enter_context(tc.tile_pool(name="psum2", bufs=3, space="PSUM"))
        cspsum = ctx.enter_context(tc.tile_pool(name="cspsum", bufs=1, space="PSUM"))

        eL = const.tile([128, _NSLOT * _BS], bf16)
        eR = const.tile([128, _NSLOT * _BS], bf16)
        posmD = const.tile([128, 2 * 4 * _W], bf16)
        negbD = const.tile([128, 2 * 4 * _W], bf16)
        corrL = const.tile([128, 2 * 128], bf16)
        corrR = const.tile([128, 2 * 128], bf16)
        samec = const.tile([128, 2 * 128], bf16)
        outs = const.tile([128, _OUTW], f32)
        colp = const.tile([128, _BS], f32)
        bm1 = const.tile([128, 1], f32)
        bm80 = const.tile([128, 1], f32)
        sel = const.tile([128, 19, 32], bf16)   # 0-14 pure, 15/16 corr-pos, 17/18 corr-neg

        nc.gpsimd.memset(outs[:], 0.0)
        nc.gpsimd.memset(bm1[:], -1.0)
        nc.gpsimd.memset(bm80[:], -80.0)
        nc.gpsimd.memset(sel[:], 0.0)
        for q in range(19):
            col = q if q < 15 else (20 + (q - 15))
            nc.gpsimd.memset(sel[:, q, col:col + 1], 1.0)

        npc = _NSLOT * _BS // 16
        for q in range(0, 16):
            nc.sync.dma_start(eL[:, q * npc:(q + 1) * npc],
                              eL_d[:, q * npc:(q + 1) * npc])
            nc.sync.dma_start(eR[:, q * npc:(q + 1) * npc],
                              eR_d[:, q * npc:(q + 1) * npc])
        nc.gpsimd.dma_start(corrL[:], corrL_d)
        nc.gpsimd.dma_start(corrR[:], corrR_d)
        nc.gpsimd.dma_start(samec[:], samec_d)
        nc.gpsimd.dma_start(posmD[:], posmD_d)
        nc.gpsimd.dma_start(negbD[:], negbD_d)

        cs = cspsum.tile([128, _BS], f32)
        cs_n = [0]
        CS_TOTAL = 60 + 4

        def colsum(selidx, ftile, w):
            cs_n[0] += 1
            nc.tensor.matmul(cs[:32, 0:w], sel[:, selidx, :], ftile,
                             start=(cs_n[0] == 1), stop=True,
                             skip_group_check=True)

        pending = []  # delayed pure colsums: (slotidx, F4 tile)

        def schrau(sq_slice, w, ocol):
            # DVE pass1+pass2: F = bitcast(A*sq+B), rowsum -> outs[:, ocol]
            it = ibpool.tile([128, _BS], i16, tag="i16s")
            nc.vector.tensor_scalar(it[:, 0:w], sq_slice, _A80, _BP,
                                    OP.mult, OP.add)
            ft = ibpool.tile([128, _BS], bf16, tag="Fs")
            nc.vector.tensor_scalar(ft[:, 0:w], it[:, 0:w].bitcast(bf16),
                                    1.0, 0.0, OP.mult, OP.add,
                                    accum_out=outs[:, ocol:ocol + 1])

        def do_corr(s):
            c0 = 128 * s
            ps2c = psum2.tile([128, 2 * _BS], f32, tag="ps2", name="ps2")
            ps = ps2c
            nc.tensor.matmul(ps[:, 0:128], corrL[:, c0:c0 + 128],
                             corrR[:, c0:c0 + 128], start=True, stop=True)
            sqc = band.tile([128, _W], bf16, tag="sqc")
            nc.scalar.activation(sqc[:, 0:128], ps[:, 0:128], AF.Square)
            itc = ibpool.tile([128, _BS], i16, tag="i16s")
            nc.vector.tensor_scalar(itc[:, 0:128], sqc[:, 0:128], _A80, _BP,
                                    OP.mult, OP.add)
            Fcm = band.tile([128, _W], bf16, tag="Fcm")
            nc.gpsimd.tensor_tensor(Fcm[:, 0:128], itc[:, 0:128].bitcast(bf16),
                                    samec[:, c0:c0 + 128], op=OP.mult)
            jc = band.tile([128, _W], bf16, tag="jc")
            nc.vector.tensor_scalar(jc[:, 0:128], Fcm[:, 0:128], 1.0, 0.0,
                                    OP.mult, OP.add,
                                    accum_out=outs[:, 92 + 2 * s:93 + 2 * s])
            colsum(17 + s, Fcm[:, 0:128], 128)
            usqc = band.tile([128, _W], bf16, tag="usqc")
            nc.scalar.activation(usqc[:, 0:128], ps[:, 0:128], AF.Square,
                                 bias=bm1[:, 0:1], scale=1.0)
            itp = ibpool.tile([128, _BS], i16, tag="i16s")
            nc.vector.tensor_scalar(itp[:, 0:128], usqc[:, 0:128], _AP80, _BPP,
                                    OP.mult, OP.add)
            itp2 = ibpool.tile([128, _BS], i16, tag="i16c")
            nc.vector.tensor_scalar(itp2[:, 0:128], itp[:, 0:128], 0.0, 0.0,
                                    OP.max, OP.add)
            Epcm = band.tile([128, _W], bf16, tag="Epcm")
            nc.gpsimd.tensor_tensor(Epcm[:, 0:128], itp2[:, 0:128].bitcast(bf16),
                                    samec[:, c0:c0 + 128], op=OP.mult)
            jc2 = band.tile([128, _W], bf16, tag="jc2")
            nc.vector.tensor_scalar(jc2[:, 0:128], Epcm[:, 0:128], 1.0, 0.0,
                                    OP.mult, OP.add,
                                    accum_out=outs[:, 93 + 2 * s:94 + 2 * s])
            colsum(15 + s, Epcm[:, 0:128], 128)

        # --- main slot loop (schedule order; eL/eR laid out in this order) ---
        for pos, k in enumerate(_SLOTORD):
            thresh = 3 if pos < 14 else 0
            while len(pending) > thresh:
                si, ftile = pending.pop(0)
                for tt in range(4):
                    colsum(si, ftile[:, tt * _BS:(tt + 1) * _BS], _BS)
            if pos == _NSLOT - 1:
                # all 64 colsums done; evacuate the colsum plane while the
                # final diag slot computes (keeps the tail off ACT's critical path)
                nc.scalar.copy(colp[0:32, :], cs[0:32, :])
                nc.sync.dma_start(colp_d, colp[0:32, :])
            r0 = pos * _BS
            if k < _NPURE:
                sq4 = sqd.tile([128, 4 * _BS], bf16, tag="sq4")
                has_d = "D" in paths[2 * k:2 * k + 2]
                if has_d:
                    sc4 = scp.tile([128, 4 * _BS], bf16, tag="sc4")
                for h in range(2):
                    path = paths[2 * k + h]
                    ps2 = psum2.tile([128, 2 * _BS], f32, tag="ps2", name="ps2")
                    for u in range(2):
                        t = 2 * h + u
                        nc.tensor.matmul(ps2[:, u * _BS:(u + 1) * _BS],
                                         eL[:, r0 + 128 * t:r0 + 128 * (t + 1)],
                                         eR[:, r0:r0 + _BS], start=True,
                                         stop=True)
                    if path == "A":
                        nc.scalar.activation(
                            sq4[:, 2 * h * _BS:2 * (h + 1) * _BS], ps2[:],
                            AF.Square)
                    else:
                        nc.vector.tensor_scalar(
                            sc4[:, 2 * h * _BS:2 * (h + 1) * _BS], ps2[:],
                            1.0, 0.0, OP.mult, OP.add)
                        nc.gpsimd.tensor_tensor(
                            sq4[:, 2 * h * _BS:2 * (h + 1) * _BS],
                            sc4[:, 2 * h * _BS:2 * (h + 1) * _BS],
                            sc4[:, 2 * h * _BS:2 * (h + 1) * _BS], op=OP.mult)
                it4 = ipool.tile([128, 4 * _BS], i16, tag="it4")
                nc.vector.tensor_scalar(it4[:], sq4[:], _A80, _BP,
                                        OP.mult, OP.add)
                ft = fpool.tile([128, 4 * _BS], bf16, tag="F4")
                for t in range(4):
                    nc.vector.tensor_scalar(
                        ft[:, t * _BS:(t + 1) * _BS],
                        it4[:, t * _BS:(t + 1) * _BS].bitcast(bf16),
                        1.0, 0.0, OP.mult, OP.add,
                        accum_out=outs[:, 4 * k + t:4 * k + t + 1])
                pending.append((k, ft))
                continue
            for h2 in range(2):
                ps2d = psum2.tile([128, 2 * _BS], f32, tag="ps2", name="ps2")
                for u2 in range(2):
                    t2 = 2 * h2 + u2
                    nc.tensor.matmul(ps2d[:, u2 * _BS:(u2 + 1) * _BS],
                                     eL[:, r0 + 128 * t2:r0 + 128 * (t2 + 1)],
                                     eR[:, r0:r0 + _BS], start=True, stop=True)
              # two diag tiles per psum2 tile
                for u2 in range(2):
                    t = 2 * h2 + u2
                    pso = u2 * _BS
                    d = k - _NPURE
                    w0 = _W0[t]
                    obase = 60 + 16 * d + 4 * t
                    moff = (d * 4 + t) * _W
                    # positives first: chain depends only on ps, and it is
                    # the longest dependency chain (matters for the drain)
                    usq = band.tile([128, _W], bf16, tag="usq")
                    nc.scalar.activation(usq[:],
                                         ps2d[:, pso + w0:pso + w0 + _W],
                                         AF.Square, bias=bm1[:, 0:1],
                                         scale=1.0)
                    itp = ibpool.tile([128, _BS], i16, tag="i16s")
                    nc.vector.tensor_scalar(itp[:, 0:_W], usq[:], _AP80, _BPP,
                                            OP.mult, OP.add)
                    itp2 = ibpool.tile([128, _BS], i16, tag="i16c")
                    nc.vector.tensor_scalar(itp2[:, 0:_W], itp[:, 0:_W], 0.0,
                                            0.0, OP.max, OP.add)
                    Epm = band.tile([128, _W], bf16, tag="Epm")
                    nc.gpsimd.tensor_tensor(Epm[:], itp2[:, 0:_W].bitcast(bf16),
                                            posmD[:, moff:moff + _W], op=OP.mult)
                    jk2 = band.tile([128, _W], bf16, tag="jk2")
                    nc.vector.tensor_scalar(jk2[:], Epm[:], 1.0, 0.0, OP.mult,
                                            OP.add,
                                            accum_out=outs[:, obase + 3:obase + 4])
                    sb = sqb.tile([128, _BS], bf16, tag="sqb")
                    nc.scalar.activation(sb[:], ps2d[:, pso:pso + _BS],
                                         AF.Square)
                    itb = ibpool.tile([128, _BS], i16, tag="i16s")
                    nc.vector.tensor_scalar(itb[:], sb[:], _A80, _BP,
                                            OP.mult, OP.add)
                    if w0 > 0:
                        fsg = ibpool.tile([128, _BS], bf16, tag="Fs")
                        nc.vector.tensor_scalar(
                            fsg[:, 0:w0], itb[:, 0:w0].bitcast(bf16), 1.0, 0.0,
                            OP.mult, OP.add,
                            accum_out=outs[:, obase + 0:obase + 1])
                    if w0 + _W < _BS:
                        fsg2 = ibpool.tile([128, _BS], bf16, tag="Fs2")
                        nc.vector.tensor_scalar(
                            fsg2[:, 0:_BS - w0 - _W],
                            itb[:, w0 + _W:].bitcast(bf16), 1.0, 0.0,
                            OP.mult, OP.add,
                            accum_out=outs[:, obase + 1:obase + 2])
                    # band negatives: schraudolph F * negb, DVE accum
                    Fbm = band.tile([128, _W], bf16, tag="Fbm")
                    nc.gpsimd.tensor_tensor(Fbm[:],
                                            itb[:, w0:w0 + _W].bitcast(bf16),
                                            negbD[:, moff:moff + _W], op=OP.mult)
                    jk = band.tile([128, _W], bf16, tag="jk")
                    nc.vector.tensor_scalar(jk[:], Fbm[:], 1.0, 0.0, OP.mult,
                                            OP.add,
                                            accum_out=outs[:, obase + 2:obase + 3])
            if pos in _CORRPOS:
                do_corr(_CORRPOS[pos])

        while pending:
            si, ftile = pending.pop(0)
            for tt in range(4):
                colsum(si, ftile[:, tt * _BS:(tt + 1) * _BS], _BS)

        nc.sync.dma_start(outs_d, outs[:])
    nc.finalize()
    return nc


def _assignment():
    """Core assignments: pure pairs, diag blocks, boundaries."""
    pairs = [(i, j) for i in range(_NB) for j in range(i + 1, _NB)]
    pure = [pairs[c::_NCORES] for c in range(_NCORES)]
    diag = [(2 * c, 2 * c + 1) for c in range(_NCORES)]
    bnds = [[] for _ in range(_NCORES)]
    for m in range(1, _NB):
        bnds[(m - 1) // 2].append(m)
    return pure, diag, bnds


def _host_prep(embeds, labels):
    import ml_dtypes
    bf = ml_dtypes.bfloat16
    labels = np.asarray(labels).astype(np.int64).ravel()
    embeds = np.asarray(embeds, dtype=np.float32)
    perm = np.argsort(labels, kind="stable")
    lab_s = labels[perm]
    emb_s = embeds[perm].astype(np.float64)
    counts = np.bincount(lab_s, minlength=int(lab_s.max()) + 1)
    assert counts.max() <= 64, f"class size {counts.max()} > 64"
    nrm = np.maximum(np.sqrt((emb_s ** 2).sum(1, keepdims=True)), 1e-12)
    eTn = np.ascontiguousarray((emb_s / nrm).T.astype(bf))  # [128, 8192]

    np_cnt = (counts[lab_s] - 1).astype(np.float64)
    nn_cnt = (_N - 1 - np_cnt).astype(np.float64)

    pure, diag, bnds = _assignment()
    in_maps = []
    for c in range(_NCORES):
        eL = np.zeros((128, _NSLOT * _BS), bf)
        eR = np.zeros((128, _NSLOT * _BS), bf)
        for pos, k in enumerate(_SLOTORD):
            if k < _NPURE:
                i, j = pure[c][k]
                bl, br = i, j
            else:
                bl = br = diag[c][k - _NPURE]
            eL[:, pos * _BS:(pos + 1) * _BS] = eTn[:, bl * _BS:(bl + 1) * _BS]
            eR[:, pos * _BS:(pos + 1) * _BS] = eTn[:, br * _BS:(br + 1) * _BS]
        posmD = np.zeros((128, 2, 4, _W), bf)
        negbD = np.zeros((128, 2, 4, _W), bf)
        for d, b in enumerate(diag[c]):
            lab_b = lab_s[b * _BS:(b + 1) * _BS]
            for t in range(4):
                w0 = _W0[t]
                la = lab_b[128 * t:128 * (t + 1)]
                lw = lab_b[w0:w0 + _W]
                same = la[:, None] == lw[None, :]
                selfc = (np.arange(w0, w0 + _W)[None, :] ==
                         (128 * t + np.arange(128))[:, None])
                posmD[:, d, t, :] = (same & ~selfc).astype(bf)
                negbD[:, d, t, :] = (~same).astype(bf)
        corrL = np.zeros((128, 2 * 128), bf)
        corrR = np.zeros((128, 2 * 128), bf)
        samec = np.zeros((128, 2 * 128), bf)
        for s, m in enumerate(bnds[c]):
            rows = slice(m * _BS - 128, m * _BS)
            cols = slice(m * _BS, m * _BS + 128)
            corrL[:, 128 * s:128 * (s + 1)] = eTn[:, rows]
            corrR[:, 128 * s:128 * (s + 1)] = eTn[:, cols]
            samec[:, 128 * s:128 * (s + 1)] = (
                lab_s[rows][:, None] == lab_s[cols][None, :]).astype(bf)
        in_maps.append({
            "eL": eL, "eR": eR,
            "posmD": np.ascontiguousarray(posmD.reshape(128, -1)),
            "negbD": np.ascontiguousarray(negbD.reshape(128, -1)),
            "corrL": corrL, "corrR": corrR, "samec": samec,
        })
    return in_maps, np_cnt, nn_cnt


def _finalize(results, np_cnt, nn_cnt):
    pure, diag, bnds = _assignment()
    neg = np.zeros(_N)
    pos = np.zeros(_N)
    for c in range(_NCORES):
        o = np.asarray(results[c]["outs"], np.float64)
        cp = np.asarray(results[c]["colp"], np.float64)
        for k, (i, j) in enumerate(pure[c]):
            for t in range(4):
                neg[i * _BS + 128 * t:i * _BS + 128 * (t + 1)] += o[:, 4 * k + t]
            neg[j * _BS:(j + 1) * _BS] += cp[k, :]
        for d, b in enumerate(diag[c]):
            for t in range(4):
                ob = 60 + 16 * d + 4 * t
                g0 = b * _BS + 128 * t
                neg[g0:g0 + 128] += o[:, ob] + o[:, ob + 1] + o[:, ob + 2]
                pos[g0:g0 + 128] += o[:, ob + 3]
        for s, m in enumerate(bnds[c]):
            neg[m * _BS - 128:m * _BS] -= o[:, 92 + 2 * s]
            pos[m * _BS - 128:m * _BS] += o[:, 93 + 2 * s]
            neg[m * _BS:m * _BS + 128] -= cp[22 + s, 0:128]
            pos[m * _BS:m * _BS + 128] += cp[20 + s, 0:128]
    valid = (np_cnt > 0) & (nn_cnt > 0) & (pos > 0) & (neg > 0)
    lse_n = 67.2 + np.log(np.where(neg > 0, neg, 1.0))
    lse_p = _CPOS + np.log(np.where(pos > 0, pos, 1.0))
    x = (lse_p + np.where(valid, np.log(nn_cnt), 0.0)
         + lse_n + np.where(valid, np.log(np_cnt), 0.0))
    sp = np.maximum(x, 0.0) + np.log1p(np.exp(-np.abs(x)))
    loss = np.where(valid, sp, 0.0).sum() / max(valid.sum(), 1)
    return np.asarray(loss, dtype=np.float32)


def kernel(embeds, labels):
    in_maps, np_cnt, nn_cnt = _host_prep(embeds, labels)
    if "nc" not in _cache:
        _cache["nc"] = _build_nc()
    from concourse.bass_utils import run_bass_kernel_spmd
    res = run_bass_kernel_spmd(_cache["nc"], in_maps,
                               core_ids=list(range(_NCORES)))
    return _finalize(res.results, np_cnt, nn_cnt)
